# revision 15
# baseline (speedup 1.0000x reference)
"""Trainium2 Bass kernel for nn_Attention_12369505813001.

Computes, per batch b:
    qw    = query @ W_in.T                      [T, H]
    score = qw @ enc.T                          [T, S]
    p     = softmax(mask(score), axis=S)
    c     = p @ enc                             [T, H]
    out   = tanh(concat(query, c) @ W_out.T + b_out)

Shapes: B=32, T=512, S=1024, H=1024, fp32. Data-parallel over B across
8 NeuronCores (4 batches/core); no collectives.

Layout strategy (per core): feature dim on partitions, T on the free
axis throughout, so the PE contraction dim always lands on partitions
and no on-device transposes are needed:
    step1  qw^T[o,t]    = W_inT-tiles(stat) @ q^T(moving)
    step2  score^T[s,t] = encT-tiles(stat)  @ qw^T(moving)
    softmax over s (partition+chunk axis): per-batch global max via
      free-axis max tree + GPSIMD partition all-reduce(max); exp on ACT
      with per-partition bias = additive length mask; denominator via a
      DVE chunk-sum tree + GPSIMD partition all-reduce(add); the
      normalization is folded into c as a broadcast mul.
    step4  c~^T[h,t]    = enc-tiles(stat)   @ e^T(moving)     fp16
    step5  out^T[o,t]   = tanh(WqT(stat) @ q^T + WcT(stat) @ cn + b)  fp16

Precision: the softmax path must be accurate — score noise of 3e-3 rms
already breaches the 2e-2 gate at near-tie softmax columns — so steps
1-2 use a split scheme at 1.5 matmul passes: a main fp16 hi*hi pass
plus ONE fp8e4m3 DoubleRow pass computing both cross terms
(lo*hi + hi*lo) at 0.5 cyc/row. Residuals lo = x - fp16(x) are
prescaled by 2^11 into fp8 range; the cross PSUM is folded back as
score = main + 2^-11 * cross on DVE. Emulated end-to-end error 3.9e-3.
Splits of q, encT, W_in are host-precomputed; qw is split on device.
Steps 4-5 are insensitive and run fully in fp16. Because e is stored
fp16 (subnormal floor ~6e-8), the softmax max MUST exclude masked
positions — a masked global max would flush every real exp to zero and
divide by a zero denominator — so the mask is folded into the max tree
(fused add+max against the per-partition mask scalar).

Schedule: software-pipelined across batches — step1(b+1) is emitted
between step2(b) and softmax(b), so the PE stays busy through the
softmax chain (which runs on DVE/ACT/GPSIMD). DMA is dominated by a
serial descriptor-generation path, so the kernel uses few, large DMA
instructions, emitted in the exact order they are consumed, with et
(encT) tiles prefetched one phase early. Output DMAs issue from the
Activation queue so their tanh-producer waits never block the input
DMA stream on SP.
"""

from contextlib import ExitStack

import numpy as np
import ml_dtypes

import concourse.bass as bass
import concourse.bass_isa as bass_isa
import concourse.mybir as mybir
import concourse.tile as tile
from concourse import bacc
from concourse.bass_utils import run_bass_kernel_spmd

B, T, S, H = 32, 512, 1024, 1024
NCORES = 8
BPC = B // NCORES          # batches per core
HT = H // 128              # h/o chunk count
ST = S // 128              # s chunk count
P = 128

f32 = mybir.dt.float32
f32r = mybir.dt.float32r
bf16 = mybir.dt.bfloat16
fp16 = mybir.dt.float16
fp8 = mybir.dt.float8e4
AF = mybir.ActivationFunctionType
ALU = mybir.AluOpType
DR = mybir.MatmulPerfMode.DoubleRow

MASKVAL = -1.0e38
RSC = 2048.0               # 2^11 residual prescale
RSCI = 1.0 / RSC

_nc_cache = []
LAST_RESULTS = None


def _build_nc():
    nc = bacc.Bacc("TRN2", target_bir_lowering=False, debug=False)

    # moving packs: [hi8, lo8'] pairs; stationary packs: [lo8', hi8]
    qhi = nc.dram_tensor("qhi", [BPC, H, T], fp16, kind="ExternalInput")
    q8 = nc.dram_tensor("q8", [BPC, 2, H, T], fp8, kind="ExternalInput")
    eThi = nc.dram_tensor("eThi", [BPC, H, S], fp16, kind="ExternalInput")
    eT8 = nc.dram_tensor("eT8", [BPC, 2, H, S], fp8, kind="ExternalInput")
    encf = nc.dram_tensor("encf", [BPC, S, H], fp16, kind="ExternalInput")
    maskc = nc.dram_tensor("maskc", [BPC, P, ST], f32, kind="ExternalInput")
    Wihi = nc.dram_tensor("Wihi", [H, H], fp16, kind="ExternalInput")  # [h,o]
    Wi8 = nc.dram_tensor("Wi8", [2, H, H], fp8, kind="ExternalInput")
    Wqf = nc.dram_tensor("Wqf", [H, H], fp16, kind="ExternalInput")
    Wcf = nc.dram_tensor("Wcf", [H, H], fp16, kind="ExternalInput")
    bo = nc.dram_tensor("bo", [P, HT], f32, kind="ExternalInput")
    outT = nc.dram_tensor("outT", [BPC, H, T], f32, kind="ExternalOutput")

    with tile.TileContext(nc) as tc, ExitStack() as ctx:
        wp = ctx.enter_context(tc.tile_pool(name="wp", bufs=1))
        pq = ctx.enter_context(tc.tile_pool(name="pq", bufs=2))
        pa = ctx.enter_context(tc.tile_pool(name="pa", bufs=2))   # qw / e
        pcs = ctx.enter_context(tc.tile_pool(name="pcs", bufs=1))  # score/cn
        tp = ctx.enter_context(tc.tile_pool(name="tp", bufs=1))
        pe1 = ctx.enter_context(tc.tile_pool(name="pe1", bufs=1))
        sp = ctx.enter_context(tc.tile_pool(name="sp", bufs=1))
        etp = ctx.enter_context(tc.tile_pool(name="etp", bufs=5))
        enp = ctx.enter_context(tc.tile_pool(name="enp", bufs=3))
        otp = ctx.enter_context(tc.tile_pool(name="otp", bufs=2))
        psA = ctx.enter_context(tc.tile_pool(name="psA", bufs=2, space="PSUM"))
        psB = ctx.enter_context(tc.tile_pool(name="psB", bufs=2, space="PSUM"))
        psC = ctx.enter_context(tc.tile_pool(name="psC", bufs=2, space="PSUM"))

        # --- persistent weights (whole-tensor DMAs; o-contiguous runs) ---
        wihi = wp.tile([P, HT, H], fp16, name="wihi")
        wi8 = wp.tile([P, 2, HT, H], fp8, name="wi8")
        wqf = wp.tile([P, HT, H], fp16, name="wqf")
        wcf = wp.tile([P, HT, H], fp16, name="wcf")
        bo_sb = wp.tile([P, HT], f32)
        mask_sb = wp.tile([P, BPC, ST], f32)

        def load_q(b):
            th = pq.tile([P, HT, T], fp16, tag="qhi", name=f"qhi_{b}")
            nc.sync.dma_start(
                out=th, in_=qhi[b].rearrange("(k p) t -> p k t", p=P))
            t8 = pq.tile([P, 2, HT, T], fp8, tag="q8", name=f"q8_{b}")
            nc.sync.dma_start(
                out=t8, in_=q8[b].rearrange("c (k p) t -> p c k t", p=P))
            return th, t8

        def emit_et(b, m):
            eh = etp.tile([P, HT, 128], fp16, tag="et", name=f"et_{b}_{m}")
            nc.sync.dma_start(
                out=eh,
                in_=eThi[b, :, 128 * m:128 * (m + 1)]
                .rearrange("(k p) s -> p k s", p=P))
            e8 = etp.tile([P, 2, HT, 128], fp8, tag="et8", name=f"et8_{b}_{m}")
            nc.sync.dma_start(
                out=e8,
                in_=eT8[b, :, :, 128 * m:128 * (m + 1)]
                .rearrange("c (k p) s -> p c k s", p=P))
            return eh, e8

        qs = {}
        qws = {}
        ets = {}

        def step1(b):
            qwhi = pa.tile([P, HT, T], fp16, tag="A", name=f"qwhi_{b}")
            qw8 = pa.tile([P, 2, HT, T], fp8, tag="A8", name=f"qw8_{b}")
            th, t8 = qs[b]
            for m in range(HT):
                msl = slice(128 * m, 128 * (m + 1))
                mp = psA.tile([P, T], f32, tag="qo", name=f"qwm_{b}_{m}")
                for k in range(HT):
                    nc.tensor.matmul(mp, wihi[:, k, msl], th[:, k, :],
                                     start=(k == 0), stop=(k == HT - 1))
                cp = psA.tile([P, T], f32, tag="qo", name=f"qwc_{b}_{m}")
                for k in range(HT):
                    nc.tensor.matmul(cp, wi8[:, :, k, msl], t8[:, :, k, :],
                                     start=(k == 0), stop=(k == HT - 1),
                                     perf_mode=DR)
                tmp = tp.tile([P, T], f32, tag="tmp")
                nc.vector.tensor_scalar_mul(tmp, cp, RSCI)
                nc.vector.tensor_add(tmp, tmp, mp)
                nc.scalar.copy(qwhi[:, m, :], tmp)
                nc.scalar.copy(qw8[:, 0, m, :], qwhi[:, m, :])
                nc.vector.tensor_sub(tmp, tmp, qwhi[:, m, :])
                nc.scalar.activation(qw8[:, 1, m, :], tmp, AF.Copy, scale=RSC)
            return qwhi, qw8

        # --- prologue ---
        nc.sync.dma_start(
            out=wihi, in_=Wihi[:, :].rearrange("(k p) o -> p k o", p=P))
        qs[0] = load_q(0)
        nc.sync.dma_start(
            out=wi8, in_=Wi8[:, :, :].rearrange("c (k p) o -> p c k o", p=P))
        nc.sync.dma_start(
            out=wqf, in_=Wqf[:, :].rearrange("(k p) o -> p k o", p=P))
        nc.sync.dma_start(
            out=wcf, in_=Wcf[:, :].rearrange("(k p) o -> p k o", p=P))
        nc.sync.dma_start(out=bo_sb, in_=bo[:, :])
        nc.sync.dma_start(out=mask_sb,
                          in_=maskc[:, :, :].rearrange("b p m -> p b m"))
        ets[0] = [emit_et(0, m) for m in range(5)]
        qws[0] = step1(0)

        for b in range(BPC):
            # --- step 2: score^T = encT @ qw^T + per-chunk max tree ---
            for m in range(5, ST):
                ets[b].append(emit_et(b, m))
            if b + 1 < BPC:
                qs[b + 1] = load_q(b + 1)
            score = pcs.tile([P, ST, T], f32, tag="B", name=f"score_{b}")
            crossb = pcs.tile([P, ST, T], bf16, tag="B2", name=f"cross_{b}")
            smax = sp.tile([P, T], f32, tag="smax")
            qwhi, qw8 = qws[b]
            for m in range(ST):
                eh, e8 = ets[b][m]
                mp = psB.tile([P, T], f32, tag="sc", name=f"scm_{b}_{m}")
                for k in range(HT):
                    nc.tensor.matmul(mp, eh[:, k, :], qwhi[:, k, :],
                                     start=(k == 0), stop=(k == HT - 1))
                cp = psB.tile([P, T], f32, tag="sc", name=f"scc_{b}_{m}")
                for k in range(HT):
                    nc.tensor.matmul(cp, e8[:, :, k, :], qw8[:, :, k, :],
                                     start=(k == 0), stop=(k == HT - 1),
                                     perf_mode=DR)
                nc.scalar.copy(score[:, m, :], mp)
                nc.vector.tensor_copy(crossb[:, m, :], cp)
                # max over UNMASKED positions only: with e stored in fp16,
                # a masked global max would flush every real exp below the
                # fp16 subnormal floor and zero the denominator.
                if m == 0:
                    nc.vector.tensor_scalar_add(smax, mp,
                                                mask_sb[:, b, m:m + 1])
                else:
                    nc.vector.scalar_tensor_tensor(smax, mp,
                                                   mask_sb[:, b, m:m + 1],
                                                   smax, ALU.add, ALU.max)

            # --- pipelined: next batch's step1 runs on PE during softmax(b) ---
            if b + 1 < BPC:
                qws[b + 1] = step1(b + 1)

            # --- softmax over s ---
            smax_all = sp.tile([P, T], f32, tag="smax_all")
            nc.gpsimd.partition_all_reduce(smax_all, smax, channels=P,
                                           reduce_op=bass_isa.ReduceOp.max)
            e = pe1.tile([P, ST, T], fp16, tag="E", name=f"e_{b}")
            for m in range(ST):
                nc.vector.scalar_tensor_tensor(score[:, m, :], crossb[:, m, :],
                                               RSCI, score[:, m, :],
                                               ALU.mult, ALU.add)
                nc.vector.tensor_sub(score[:, m, :], score[:, m, :], smax_all)
                nc.scalar.activation(e[:, m, :], score[:, m, :], AF.Exp,
                                     bias=mask_sb[:, b, m:m + 1])
            esum = sp.tile([P, T], f32, tag="smax")
            nc.vector.tensor_add(esum, e[:, 0, :], e[:, 1, :])
            for m in range(2, ST):
                nc.vector.tensor_add(esum, esum, e[:, m, :])
            esum_all = sp.tile([P, T], f32, tag="esum_all")
            nc.gpsimd.partition_all_reduce(esum_all, esum, channels=P,
                                           reduce_op=bass_isa.ReduceOp.add)
            rdenb = sp.tile([P, T], f32, tag="smax_all")
            nc.vector.reciprocal(rdenb, esum_all)

            # --- step 4: c~^T = enc @ e^T (fp32r), fold in 1/denom ---
            cn = pcs.tile([P, HT, T], fp16, tag="B2", name=f"cn_{b}")
            for m in range(HT):
                en = enp.tile([P, ST, 128], fp16, tag="en", name=f"en_{b}_{m}")
                nc.sync.dma_start(
                    out=en,
                    in_=encf[b, :, 128 * m:128 * (m + 1)]
                    .rearrange("(k p) h -> p k h", p=P))
                c_ps = psC.tile([P, T], f32, tag="c", name=f"c_{b}_{m}")
                for k in range(ST):
                    nc.tensor.matmul(c_ps, en[:, k, :], e[:, k, :],
                                     start=(k == 0), stop=(k == ST - 1))
                nc.vector.tensor_mul(cn[:, m, :], c_ps, rdenb)

            # --- step 5: out^T = tanh(WqT @ q^T + WcT @ cn + b), fp16 ---
            if b + 1 < BPC:
                ets[b + 1] = [emit_et(b + 1, m) for m in range(5)]
            for m in range(HT):
                o_ps = psA.tile([P, T], f32, tag="qo", name=f"o_{b}_{m}")
                msl = slice(128 * m, 128 * (m + 1))
                for k in range(HT):
                    nc.tensor.matmul(o_ps, wqf[:, k, msl], qs[b][0][:, k, :],
                                     start=(k == 0), stop=False)
                for k in range(HT):
                    nc.tensor.matmul(o_ps, wcf[:, k, msl], cn[:, k, :],
                                     start=False, stop=(k == HT - 1))
                ot = otp.tile([P, T], f32, tag="ot")
                nc.scalar.activation(ot, o_ps, AF.Tanh, bias=bo_sb[:, m:m + 1])
                nc.scalar.dma_start(out=outT[b, 128 * m:128 * (m + 1), :],
                                    in_=ot)

    nc.compile()
    return nc


def _split16(x):
    """fp16 hi + fp8 pack [hi8, 2^11*lo in fp8] (moving order)."""
    hi = x.astype(np.float16)
    lo = (x - hi.astype(np.float32)) * RSC
    return hi, hi.astype(ml_dtypes.float8_e4m3), lo.astype(ml_dtypes.float8_e4m3)


def kernel(query, encoder_outputs, src_lengths, W_in, W_out, b_out):
    query = np.asarray(query, dtype=np.float32)
    encoder_outputs = np.ascontiguousarray(np.asarray(encoder_outputs, np.float32))
    src_lengths = np.asarray(src_lengths)
    W_in = np.asarray(W_in, dtype=np.float32)
    W_out = np.asarray(W_out, dtype=np.float32)
    b_out = np.asarray(b_out, dtype=np.float32)

    # --- shared (weight) inputs ---
    W_inT = np.ascontiguousarray(W_in.T)                    # [h, o]
    Wihi, Wih8, Wil8 = _split16(W_inT)
    Wi8 = np.ascontiguousarray(np.stack([Wil8, Wih8], axis=0))  # stat: [lo, hi]
    Wqf = np.ascontiguousarray(W_out[:, :H].T).astype(np.float16)
    Wcf = np.ascontiguousarray(W_out[:, H:].T).astype(np.float16)
    bo = np.ascontiguousarray(b_out.reshape(HT, P).T)       # [p, m]

    # --- per-core shards ---
    in_maps = []
    for c in range(NCORES):
        bs = slice(c * BPC, (c + 1) * BPC)
        q = query[bs]                                       # [BPC, T, H]
        encs = encoder_outputs[bs]                          # [BPC, S, H]
        lens = np.asarray(src_lengths[bs], dtype=np.int64)

        qTa = np.ascontiguousarray(q.transpose(0, 2, 1))    # [BPC, H, T]
        qh, qh8, ql8 = _split16(qTa)
        q8a = np.ascontiguousarray(np.stack([qh8, ql8], axis=1))  # mov: [hi, lo]
        eTa = np.ascontiguousarray(encs.transpose(0, 2, 1))  # [BPC, H, S]
        eh, eh8, el8 = _split16(eTa)
        eT8a = np.ascontiguousarray(np.stack([el8, eh8], axis=1))  # stat: [lo, hi]

        maskca = np.zeros((BPC, P, ST), dtype=np.float32)
        pos = (np.arange(ST)[None, :] * P + np.arange(P)[:, None])  # [P, ST]
        for j in range(BPC):
            maskca[j][pos >= lens[j]] = MASKVAL

        in_maps.append({
            "qhi": qh, "q8": q8a, "eThi": eh, "eT8": eT8a,
            "encf": encs.astype(np.float16),
            "maskc": maskca, "Wihi": Wihi, "Wi8": Wi8,
            "Wqf": Wqf, "Wcf": Wcf, "bo": bo,
        })

    if not _nc_cache:
        _nc_cache.append(_build_nc())
    nc = _nc_cache[0]

    res = run_bass_kernel_spmd(nc, in_maps, core_ids=list(range(NCORES)))
    global LAST_RESULTS
    LAST_RESULTS = res

    out = np.empty((B, T, H), dtype=np.float32)
    for c in range(NCORES):
        o = res.results[c]["outT"]                          # [BPC, H, T]
        out[c * BPC:(c + 1) * BPC] = o.transpose(0, 2, 1)
    return out


# revision 16
# speedup vs baseline: 1.0127x; 1.0127x over previous
"""Trainium2 Bass kernel for nn_Attention_12369505813001.

Computes, per batch b:
    qw    = query @ W_in.T                      [T, H]
    score = qw @ enc.T                          [T, S]
    p     = softmax(mask(score), axis=S)
    c     = p @ enc                             [T, H]
    out   = tanh(concat(query, c) @ W_out.T + b_out)

Shapes: B=32, T=512, S=1024, H=1024, fp32. Data-parallel over B across
8 NeuronCores (4 batches/core); no collectives.

Layout strategy (per core): feature dim on partitions, T on the free
axis throughout, so the PE contraction dim always lands on partitions
and no on-device transposes are needed:
    step1  qw^T[o,t]    = W_inT-tiles(stat) @ q^T(moving)
    step2  score^T[s,t] = encT-tiles(stat)  @ qw^T(moving)
    softmax over s (partition+chunk axis): per-batch global max via
      free-axis max tree + GPSIMD partition all-reduce(max); exp on ACT
      with per-partition bias = additive length mask; denominator via a
      DVE chunk-sum tree + GPSIMD partition all-reduce(add); the
      normalization is folded into c as a broadcast mul.
    step4  c~^T[h,t]    = enc-tiles(stat)   @ e^T(moving)     fp16
    step5  out^T[o,t]   = tanh(WqT(stat) @ q^T + WcT(stat) @ cn + b)  fp16

Precision: the softmax path must be accurate — score noise of 3e-3 rms
already breaches the 2e-2 gate at near-tie softmax columns — so steps
1-2 use a split scheme at 1.5 matmul passes: a main fp16 hi*hi pass
plus ONE fp8e4m3 DoubleRow pass computing both cross terms
(lo*hi + hi*lo) at 0.5 cyc/row. Residuals lo = x - fp16(x) are
prescaled by 2^11 into fp8 range; the cross PSUM is folded back as
score = main + 2^-11 * cross on DVE. Emulated end-to-end error 3.9e-3.
Splits of q, encT, W_in are host-precomputed; qw is split on device.
Steps 4-5 are insensitive and run fully in fp16. Because e is stored
fp16 (subnormal floor ~6e-8), the softmax max MUST exclude masked
positions — a masked global max would flush every real exp to zero and
divide by a zero denominator — so the mask is folded into the max tree
(fused add+max against the per-partition mask scalar).

Schedule: software-pipelined across batches — step1(b+1) is emitted
between step2(b) and softmax(b), so the PE stays busy through the
softmax chain (which runs on DVE/ACT/GPSIMD). DMA is dominated by a
serial descriptor-generation path, so the kernel uses few, large DMA
instructions, emitted in the exact order they are consumed, with et
(encT) tiles prefetched one phase early. Output DMAs issue from the
Activation queue so their tanh-producer waits never block the input
DMA stream on SP.
"""

from contextlib import ExitStack

import numpy as np
import ml_dtypes

import concourse.bass as bass
import concourse.bass_isa as bass_isa
import concourse.mybir as mybir
import concourse.tile as tile
from concourse import bacc
from concourse.bass_utils import run_bass_kernel_spmd

B, T, S, H = 32, 512, 1024, 1024
NCORES = 8
BPC = B // NCORES          # batches per core
HT = H // 128              # h/o chunk count
ST = S // 128              # s chunk count
P = 128

f32 = mybir.dt.float32
f32r = mybir.dt.float32r
bf16 = mybir.dt.bfloat16
fp16 = mybir.dt.float16
fp8 = mybir.dt.float8e4
AF = mybir.ActivationFunctionType
ALU = mybir.AluOpType
DR = mybir.MatmulPerfMode.DoubleRow

MASKVAL = -1.0e38
RSC = 2048.0               # 2^11 residual prescale
RSCI = 1.0 / RSC

_nc_cache = []
LAST_RESULTS = None


def _build_nc():
    nc = bacc.Bacc("TRN2", target_bir_lowering=False, debug=False)

    # moving packs: [hi8, lo8'] pairs; stationary packs: [lo8', hi8]
    qhi = nc.dram_tensor("qhi", [BPC, H, T], fp16, kind="ExternalInput")
    q8 = nc.dram_tensor("q8", [BPC, 2, H, T], fp8, kind="ExternalInput")
    eThi = nc.dram_tensor("eThi", [BPC, H, S], fp16, kind="ExternalInput")
    eT8 = nc.dram_tensor("eT8", [BPC, 2, H, S], fp8, kind="ExternalInput")
    encf = nc.dram_tensor("encf", [BPC, S, H], fp16, kind="ExternalInput")
    maskc = nc.dram_tensor("maskc", [BPC, P, ST], f32, kind="ExternalInput")
    Wihi = nc.dram_tensor("Wihi", [H, H], fp16, kind="ExternalInput")  # [h,o]
    Wi8 = nc.dram_tensor("Wi8", [2, H, H], fp8, kind="ExternalInput")
    Wqf = nc.dram_tensor("Wqf", [H, H], fp16, kind="ExternalInput")
    Wcf = nc.dram_tensor("Wcf", [H, H], fp16, kind="ExternalInput")
    bo = nc.dram_tensor("bo", [P, HT], f32, kind="ExternalInput")
    outT = nc.dram_tensor("outT", [BPC, H, T], f32, kind="ExternalOutput")

    with tile.TileContext(nc) as tc, ExitStack() as ctx:
        wp = ctx.enter_context(tc.tile_pool(name="wp", bufs=1))
        pq = ctx.enter_context(tc.tile_pool(name="pq", bufs=2))
        pa = ctx.enter_context(tc.tile_pool(name="pa", bufs=2))   # qw / e
        pcs = ctx.enter_context(tc.tile_pool(name="pcs", bufs=1))  # score/cn
        tp = ctx.enter_context(tc.tile_pool(name="tp", bufs=1))
        pe1 = ctx.enter_context(tc.tile_pool(name="pe1", bufs=1))
        sp = ctx.enter_context(tc.tile_pool(name="sp", bufs=1))
        etp = ctx.enter_context(tc.tile_pool(name="etp", bufs=6))
        enp = ctx.enter_context(tc.tile_pool(name="enp", bufs=3))
        otp = ctx.enter_context(tc.tile_pool(name="otp", bufs=2))
        psA = ctx.enter_context(tc.tile_pool(name="psA", bufs=2, space="PSUM"))
        psB = ctx.enter_context(tc.tile_pool(name="psB", bufs=2, space="PSUM"))
        psC = ctx.enter_context(tc.tile_pool(name="psC", bufs=2, space="PSUM"))

        # --- persistent weights (whole-tensor DMAs; o-contiguous runs) ---
        wihi = wp.tile([P, HT, H], fp16, name="wihi")
        wi8 = wp.tile([P, 2, HT, H], fp8, name="wi8")
        wqf = wp.tile([P, HT, H], fp16, name="wqf")
        wcf = wp.tile([P, HT, H], fp16, name="wcf")
        bo_sb = wp.tile([P, HT], f32)
        mask_sb = wp.tile([P, BPC, ST], f32)

        def load_q(b):
            th = pq.tile([P, HT, T], fp16, tag="qhi", name=f"qhi_{b}")
            nc.sync.dma_start(
                out=th, in_=qhi[b].rearrange("(k p) t -> p k t", p=P))
            t8 = pq.tile([P, 2, HT, T], fp8, tag="q8", name=f"q8_{b}")
            nc.sync.dma_start(
                out=t8, in_=q8[b].rearrange("c (k p) t -> p c k t", p=P))
            return th, t8

        def emit_et(b, m):
            eh = etp.tile([P, HT, 128], fp16, tag="et", name=f"et_{b}_{m}")
            nc.sync.dma_start(
                out=eh,
                in_=eThi[b, :, 128 * m:128 * (m + 1)]
                .rearrange("(k p) s -> p k s", p=P))
            e8 = etp.tile([P, 2, HT, 128], fp8, tag="et8", name=f"et8_{b}_{m}")
            nc.sync.dma_start(
                out=e8,
                in_=eT8[b, :, :, 128 * m:128 * (m + 1)]
                .rearrange("c (k p) s -> p c k s", p=P))
            return eh, e8

        qs = {}
        qws = {}
        ets = {}

        def step1(b):
            qwhi = pa.tile([P, HT, T], fp16, tag="A", name=f"qwhi_{b}")
            qw8 = pa.tile([P, 2, HT, T], fp8, tag="A8", name=f"qw8_{b}")
            th, t8 = qs[b]
            for m in range(HT):
                msl = slice(128 * m, 128 * (m + 1))
                mp = psA.tile([P, T], f32, tag="qo", name=f"qwm_{b}_{m}")
                for k in range(HT):
                    nc.tensor.matmul(mp, wihi[:, k, msl], th[:, k, :],
                                     start=(k == 0), stop=(k == HT - 1))
                cp = psA.tile([P, T], f32, tag="qo", name=f"qwc_{b}_{m}")
                for k in range(HT):
                    nc.tensor.matmul(cp, wi8[:, :, k, msl], t8[:, :, k, :],
                                     start=(k == 0), stop=(k == HT - 1),
                                     perf_mode=DR)
                tmp = tp.tile([P, T], f32, tag="tmp")
                nc.vector.tensor_scalar_mul(tmp, cp, RSCI)
                nc.vector.tensor_add(tmp, tmp, mp)
                nc.scalar.copy(qwhi[:, m, :], tmp)
                nc.scalar.copy(qw8[:, 0, m, :], qwhi[:, m, :])
                nc.vector.tensor_sub(tmp, tmp, qwhi[:, m, :])
                nc.scalar.activation(qw8[:, 1, m, :], tmp, AF.Copy, scale=RSC)
            return qwhi, qw8

        # --- prologue ---
        # weight halves interleaved with q so step1(0) passes start early
        nc.sync.dma_start(
            out=wihi[:, :, 0:512],
            in_=Wihi[:, 0:512].rearrange("(k p) o -> p k o", p=P))
        qs[0] = load_q(0)
        nc.sync.dma_start(
            out=wi8[:, :, :, 0:512],
            in_=Wi8[:, :, 0:512].rearrange("c (k p) o -> p c k o", p=P))
        nc.sync.dma_start(
            out=wihi[:, :, 512:1024],
            in_=Wihi[:, 512:1024].rearrange("(k p) o -> p k o", p=P))
        nc.sync.dma_start(
            out=wi8[:, :, :, 512:1024],
            in_=Wi8[:, :, 512:1024].rearrange("c (k p) o -> p c k o", p=P))
        nc.sync.dma_start(
            out=wqf, in_=Wqf[:, :].rearrange("(k p) o -> p k o", p=P))
        nc.sync.dma_start(
            out=wcf, in_=Wcf[:, :].rearrange("(k p) o -> p k o", p=P))
        nc.sync.dma_start(out=bo_sb, in_=bo[:, :])
        nc.sync.dma_start(out=mask_sb,
                          in_=maskc[:, :, :].rearrange("b p m -> p b m"))
        ets[0] = [emit_et(0, m) for m in range(6)]
        qws[0] = step1(0)

        for b in range(BPC):
            # --- step 2: score^T = encT @ qw^T + per-chunk max tree ---
            for m in range(6, ST):
                ets[b].append(emit_et(b, m))
            if b + 1 < BPC:
                qs[b + 1] = load_q(b + 1)
            score = pcs.tile([P, ST, T], f32, tag="B", name=f"score_{b}")
            crossb = pcs.tile([P, ST, T], bf16, tag="B2", name=f"cross_{b}")
            smax = sp.tile([P, T], f32, tag="smax")
            qwhi, qw8 = qws[b]
            for m in range(ST):
                eh, e8 = ets[b][m]
                mp = psB.tile([P, T], f32, tag="sc", name=f"scm_{b}_{m}")
                for k in range(HT):
                    nc.tensor.matmul(mp, eh[:, k, :], qwhi[:, k, :],
                                     start=(k == 0), stop=(k == HT - 1))
                cp = psB.tile([P, T], f32, tag="sc", name=f"scc_{b}_{m}")
                for k in range(HT):
                    nc.tensor.matmul(cp, e8[:, :, k, :], qw8[:, :, k, :],
                                     start=(k == 0), stop=(k == HT - 1),
                                     perf_mode=DR)
                nc.scalar.copy(score[:, m, :], mp)
                nc.vector.tensor_copy(crossb[:, m, :], cp)
                # max over UNMASKED positions only: with e stored in fp16,
                # a masked global max would flush every real exp below the
                # fp16 subnormal floor and zero the denominator.
                if m == 0:
                    nc.vector.tensor_scalar_add(smax, mp,
                                                mask_sb[:, b, m:m + 1])
                else:
                    nc.vector.scalar_tensor_tensor(smax, mp,
                                                   mask_sb[:, b, m:m + 1],
                                                   smax, ALU.add, ALU.max)

            # --- pipelined: next batch's step1 runs on PE during softmax(b) ---
            if b + 1 < BPC:
                qws[b + 1] = step1(b + 1)

            # --- softmax over s ---
            smax_all = sp.tile([P, T], f32, tag="smax_all")
            nc.gpsimd.partition_all_reduce(smax_all, smax, channels=P,
                                           reduce_op=bass_isa.ReduceOp.max)
            e = pe1.tile([P, ST, T], fp16, tag="E", name=f"e_{b}")
            for m in range(ST):
                nc.vector.scalar_tensor_tensor(score[:, m, :], crossb[:, m, :],
                                               RSCI, score[:, m, :],
                                               ALU.mult, ALU.add)
                nc.vector.tensor_sub(score[:, m, :], score[:, m, :], smax_all)
                nc.scalar.activation(e[:, m, :], score[:, m, :], AF.Exp,
                                     bias=mask_sb[:, b, m:m + 1])
            esum = sp.tile([P, T], f32, tag="smax")
            nc.vector.tensor_add(esum, e[:, 0, :], e[:, 1, :])
            for m in range(2, ST):
                nc.vector.tensor_add(esum, esum, e[:, m, :])
            esum_all = sp.tile([P, T], f32, tag="esum_all")
            nc.gpsimd.partition_all_reduce(esum_all, esum, channels=P,
                                           reduce_op=bass_isa.ReduceOp.add)
            rdenb = sp.tile([P, T], f32, tag="smax_all")
            nc.vector.reciprocal(rdenb, esum_all)

            # --- step 4: c~^T = enc @ e^T (fp32r), fold in 1/denom ---
            cn = pcs.tile([P, HT, T], fp16, tag="B2", name=f"cn_{b}")
            for m in range(HT):
                en = enp.tile([P, ST, 128], fp16, tag="en", name=f"en_{b}_{m}")
                nc.sync.dma_start(
                    out=en,
                    in_=encf[b, :, 128 * m:128 * (m + 1)]
                    .rearrange("(k p) h -> p k h", p=P))
                c_ps = psC.tile([P, T], f32, tag="c", name=f"c_{b}_{m}")
                for k in range(ST):
                    nc.tensor.matmul(c_ps, en[:, k, :], e[:, k, :],
                                     start=(k == 0), stop=(k == ST - 1))
                nc.vector.tensor_mul(cn[:, m, :], c_ps, rdenb)

            # --- step 5: out^T = tanh(WqT @ q^T + WcT @ cn + b), fp16 ---
            if b + 1 < BPC:
                ets[b + 1] = [emit_et(b + 1, m) for m in range(6)]
            for m in range(HT):
                o_ps = psA.tile([P, T], f32, tag="qo", name=f"o_{b}_{m}")
                msl = slice(128 * m, 128 * (m + 1))
                for k in range(HT):
                    nc.tensor.matmul(o_ps, wqf[:, k, msl], qs[b][0][:, k, :],
                                     start=(k == 0), stop=False)
                for k in range(HT):
                    nc.tensor.matmul(o_ps, wcf[:, k, msl], cn[:, k, :],
                                     start=False, stop=(k == HT - 1))
                ot = otp.tile([P, T], f32, tag="ot")
                nc.scalar.activation(ot, o_ps, AF.Tanh, bias=bo_sb[:, m:m + 1])
                nc.scalar.dma_start(out=outT[b, 128 * m:128 * (m + 1), :],
                                    in_=ot)

    nc.compile()
    return nc


def _split16(x):
    """fp16 hi + fp8 pack [hi8, 2^11*lo in fp8] (moving order)."""
    hi = x.astype(np.float16)
    lo = (x - hi.astype(np.float32)) * RSC
    return hi, hi.astype(ml_dtypes.float8_e4m3), lo.astype(ml_dtypes.float8_e4m3)


def kernel(query, encoder_outputs, src_lengths, W_in, W_out, b_out):
    query = np.asarray(query, dtype=np.float32)
    encoder_outputs = np.ascontiguousarray(np.asarray(encoder_outputs, np.float32))
    src_lengths = np.asarray(src_lengths)
    W_in = np.asarray(W_in, dtype=np.float32)
    W_out = np.asarray(W_out, dtype=np.float32)
    b_out = np.asarray(b_out, dtype=np.float32)

    # --- shared (weight) inputs ---
    W_inT = np.ascontiguousarray(W_in.T)                    # [h, o]
    Wihi, Wih8, Wil8 = _split16(W_inT)
    Wi8 = np.ascontiguousarray(np.stack([Wil8, Wih8], axis=0))  # stat: [lo, hi]
    Wqf = np.ascontiguousarray(W_out[:, :H].T).astype(np.float16)
    Wcf = np.ascontiguousarray(W_out[:, H:].T).astype(np.float16)
    bo = np.ascontiguousarray(b_out.reshape(HT, P).T)       # [p, m]

    # --- per-core shards ---
    in_maps = []
    for c in range(NCORES):
        bs = slice(c * BPC, (c + 1) * BPC)
        q = query[bs]                                       # [BPC, T, H]
        encs = encoder_outputs[bs]                          # [BPC, S, H]
        lens = np.asarray(src_lengths[bs], dtype=np.int64)

        qTa = np.ascontiguousarray(q.transpose(0, 2, 1))    # [BPC, H, T]
        qh, qh8, ql8 = _split16(qTa)
        q8a = np.ascontiguousarray(np.stack([qh8, ql8], axis=1))  # mov: [hi, lo]
        eTa = np.ascontiguousarray(encs.transpose(0, 2, 1))  # [BPC, H, S]
        eh, eh8, el8 = _split16(eTa)
        eT8a = np.ascontiguousarray(np.stack([el8, eh8], axis=1))  # stat: [lo, hi]

        maskca = np.zeros((BPC, P, ST), dtype=np.float32)
        pos = (np.arange(ST)[None, :] * P + np.arange(P)[:, None])  # [P, ST]
        for j in range(BPC):
            maskca[j][pos >= lens[j]] = MASKVAL

        in_maps.append({
            "qhi": qh, "q8": q8a, "eThi": eh, "eT8": eT8a,
            "encf": encs.astype(np.float16),
            "maskc": maskca, "Wihi": Wihi, "Wi8": Wi8,
            "Wqf": Wqf, "Wcf": Wcf, "bo": bo,
        })

    if not _nc_cache:
        _nc_cache.append(_build_nc())
    nc = _nc_cache[0]

    res = run_bass_kernel_spmd(nc, in_maps, core_ids=list(range(NCORES)))
    global LAST_RESULTS
    LAST_RESULTS = res

    out = np.empty((B, T, H), dtype=np.float32)
    for c in range(NCORES):
        o = res.results[c]["outT"]                          # [BPC, H, T]
        out[c * BPC:(c + 1) * BPC] = o.transpose(0, 2, 1)
    return out


# revision 21
# speedup vs baseline: 1.0597x; 1.0463x over previous
"""Trainium2 Bass kernel for nn_Attention_12369505813001.

Computes, per batch b:
    qw    = query @ W_in.T                      [T, H]
    score = qw @ enc.T                          [T, S]
    p     = softmax(mask(score), axis=S)
    c     = p @ enc                             [T, H]
    out   = tanh(concat(query, c) @ W_out.T + b_out)

Shapes: B=32, T=512, S=1024, H=1024, fp32. Data-parallel over B across
8 NeuronCores (4 batches/core); no collectives.

Layout strategy (per core): feature dim on partitions, T on the free
axis throughout, so the PE contraction dim always lands on partitions
and no on-device transposes are needed:
    step1  qw^T[o,t]    = W_inT-tiles(stat) @ q^T(moving)
    step2  score^T[s,t] = encT-tiles(stat)  @ qw^T(moving)
    softmax over s (partition+chunk axis): per-batch global max via
      free-axis max tree + GPSIMD partition all-reduce(max); exp on ACT
      with per-partition bias = additive length mask; denominator via a
      DVE chunk-sum tree + GPSIMD partition all-reduce(add); the
      normalization is folded into c as a broadcast mul.
    step4  c~^T[h,t]    = enc-tiles(stat)   @ e^T(moving)     fp16
    step5  out^T[o,t]   = tanh(WqT(stat) @ q^T + WcT(stat) @ cn + b)  fp16

Precision: the softmax path must be accurate — score noise of 3e-3 rms
already breaches the 2e-2 gate at near-tie softmax columns — so steps
1-2 use a split scheme at 1.5 matmul passes: a main fp16 hi*hi pass
plus ONE fp8e4m3 DoubleRow pass computing both cross terms
(lo*hi + hi*lo) at 0.5 cyc/row. Residuals lo = x - fp16(x) are
prescaled by 2^11 into fp8 range; the cross PSUM is folded back as
score = main + 2^-11 * cross on DVE. Emulated end-to-end error 3.9e-3.
Splits of q, encT, W_in are host-precomputed; qw is split on device.
Steps 4-5 are insensitive and run fully in fp16. Because e is stored
fp16 (subnormal floor ~6e-8), the softmax max MUST exclude masked
positions — a masked global max would flush every real exp to zero and
divide by a zero denominator — so the mask is folded into the max tree
(fused add+max against the per-partition mask scalar).

Schedule: software-pipelined across batches — step1(b+1) is emitted
between step2(b) and softmax(b), so the PE stays busy through the
softmax chain (which runs on DVE/ACT/GPSIMD). DMA is dominated by a
serial descriptor-generation path, so the kernel uses few, large DMA
instructions, emitted in the exact order they are consumed, with et
(encT) tiles prefetched one phase early. Output DMAs issue from the
Activation queue so their tanh-producer waits never block the input
DMA stream on SP.
"""

from contextlib import ExitStack

import numpy as np
import ml_dtypes

import concourse.bass as bass
import concourse.bass_isa as bass_isa
import concourse.mybir as mybir
import concourse.tile as tile
from concourse import bacc
from concourse.bass_utils import run_bass_kernel_spmd

B, T, S, H = 32, 512, 1024, 1024
NCORES = 8
BPC = B // NCORES          # batches per core
HT = H // 128              # h/o chunk count
ST = S // 128              # s chunk count
P = 128

f32 = mybir.dt.float32
f32r = mybir.dt.float32r
bf16 = mybir.dt.bfloat16
fp16 = mybir.dt.float16
fp8 = mybir.dt.float8e4
AF = mybir.ActivationFunctionType
ALU = mybir.AluOpType
DR = mybir.MatmulPerfMode.DoubleRow

MASKVAL = -1.0e38
RSC = 2048.0               # 2^11 residual prescale
RSCI = 1.0 / RSC

_nc_cache = []
LAST_RESULTS = None


def _build_nc():
    nc = bacc.Bacc("TRN2", target_bir_lowering=False, debug=False)

    # moving packs: [hi8, lo8'] pairs; stationary packs: [lo8', hi8]
    qhi = nc.dram_tensor("qhi", [BPC, H, T], fp16, kind="ExternalInput")
    q8 = nc.dram_tensor("q8", [BPC, 2, H, T], fp8, kind="ExternalInput")
    eThi = nc.dram_tensor("eThi", [BPC, H, S], fp16, kind="ExternalInput")
    eT8 = nc.dram_tensor("eT8", [BPC, 2, H, S], fp8, kind="ExternalInput")
    encf = nc.dram_tensor("encf", [BPC, S, H], fp16, kind="ExternalInput")
    maskc = nc.dram_tensor("maskc", [BPC, P, ST], f32, kind="ExternalInput")
    Wihi = nc.dram_tensor("Wihi", [H, H], fp16, kind="ExternalInput")  # [h,o]
    Wi8 = nc.dram_tensor("Wi8", [2, H, H], fp8, kind="ExternalInput")
    Wqf = nc.dram_tensor("Wqf", [H, H], fp16, kind="ExternalInput")
    Wcf = nc.dram_tensor("Wcf", [H, H], fp16, kind="ExternalInput")
    bo = nc.dram_tensor("bo", [P, HT], f32, kind="ExternalInput")
    outT = nc.dram_tensor("outT", [BPC, H, T], f32, kind="ExternalOutput")

    with tile.TileContext(nc) as tc, ExitStack() as ctx:
        wp = ctx.enter_context(tc.tile_pool(name="wp", bufs=1))
        pq = ctx.enter_context(tc.tile_pool(name="pq", bufs=2))
        pa = ctx.enter_context(tc.tile_pool(name="pa", bufs=2))   # qw / e
        pcs = ctx.enter_context(tc.tile_pool(name="pcs", bufs=1))  # score/cn
        tp = ctx.enter_context(tc.tile_pool(name="tp", bufs=1))
        pe1 = ctx.enter_context(tc.tile_pool(name="pe1", bufs=1))
        sp = ctx.enter_context(tc.tile_pool(name="sp", bufs=1))
        etp = ctx.enter_context(tc.tile_pool(name="etp", bufs=6))
        enp = ctx.enter_context(tc.tile_pool(name="enp", bufs=3))
        otp = ctx.enter_context(tc.tile_pool(name="otp", bufs=2))
        psA = ctx.enter_context(tc.tile_pool(name="psA", bufs=2, space="PSUM"))
        psB = ctx.enter_context(tc.tile_pool(name="psB", bufs=2, space="PSUM"))
        psC = ctx.enter_context(tc.tile_pool(name="psC", bufs=2, space="PSUM"))

        # --- persistent weights (whole-tensor DMAs; o-contiguous runs) ---
        wihi = wp.tile([P, HT, H], fp16, name="wihi")
        wi8 = wp.tile([P, 2, HT, H], fp8, name="wi8")
        wqf = wp.tile([P, HT, H], fp16, name="wqf")
        wcf = wp.tile([P, HT, H], fp16, name="wcf")
        bo_sb = wp.tile([P, HT], f32)
        mask_sb = wp.tile([P, BPC, ST], f32)

        def load_q(b):
            th = pq.tile([P, HT, T], fp16, tag="qhi", name=f"qhi_{b}")
            nc.sync.dma_start(
                out=th, in_=qhi[b].rearrange("(k p) t -> p k t", p=P))
            t8 = pq.tile([P, 2, HT, T], fp8, tag="q8", name=f"q8_{b}")
            nc.sync.dma_start(
                out=t8, in_=q8[b].rearrange("c (k p) t -> p c k t", p=P))
            return th, t8

        def emit_et(b, m):
            eh = etp.tile([P, HT, 128], fp16, tag="et", name=f"et_{b}_{m}")
            nc.sync.dma_start(
                out=eh,
                in_=eThi[b, :, 128 * m:128 * (m + 1)]
                .rearrange("(k p) s -> p k s", p=P))
            e8 = etp.tile([P, 2, HT, 128], fp8, tag="et8", name=f"et8_{b}_{m}")
            nc.sync.dma_start(
                out=e8,
                in_=eT8[b, :, :, 128 * m:128 * (m + 1)]
                .rearrange("c (k p) s -> p c k s", p=P))
            return eh, e8

        qs = {}
        qws = {}
        ets = {}

        def step1(b):
            qwhi = pa.tile([P, HT, T], fp16, tag="A", name=f"qwhi_{b}")
            qw8 = pa.tile([P, 2, HT, T], fp8, tag="A8", name=f"qw8_{b}")
            th, t8 = qs[b]
            for m in range(HT):
                msl = slice(128 * m, 128 * (m + 1))
                mp = psA.tile([P, T], f32, tag="qo", name=f"qwm_{b}_{m}")
                for k in range(HT):
                    nc.tensor.matmul(mp, wihi[:, k, msl], th[:, k, :],
                                     start=(k == 0), stop=(k == HT - 1))
                cp = psA.tile([P, T], f32, tag="qo", name=f"qwc_{b}_{m}")
                for k in range(HT):
                    nc.tensor.matmul(cp, wi8[:, :, k, msl], t8[:, :, k, :],
                                     start=(k == 0), stop=(k == HT - 1),
                                     perf_mode=DR)
                tmp = tp.tile([P, T], f32, tag="tmp")
                nc.vector.tensor_scalar_mul(tmp, cp, RSCI)
                nc.vector.tensor_add(tmp, tmp, mp)
                nc.scalar.copy(qwhi[:, m, :], tmp)
                nc.scalar.copy(qw8[:, 0, m, :], qwhi[:, m, :])
                nc.vector.tensor_sub(tmp, tmp, qwhi[:, m, :])
                nc.scalar.activation(qw8[:, 1, m, :], tmp, AF.Copy, scale=RSC)
            return qwhi, qw8

        # --- prologue ---
        # weight slices interleaved with q so step1(0) passes start early
        nc.sync.dma_start(
            out=wihi[:, :, 0:128],
            in_=Wihi[:, 0:128].rearrange("(k p) o -> p k o", p=P))
        qs[0] = load_q(0)
        nc.sync.dma_start(
            out=wi8[:, :, :, 0:128],
            in_=Wi8[:, :, 0:128].rearrange("c (k p) o -> p c k o", p=P))
        nc.sync.dma_start(
            out=wihi[:, :, 128:512],
            in_=Wihi[:, 128:512].rearrange("(k p) o -> p k o", p=P))
        nc.sync.dma_start(
            out=wi8[:, :, :, 128:512],
            in_=Wi8[:, :, 128:512].rearrange("c (k p) o -> p c k o", p=P))
        nc.sync.dma_start(
            out=wihi[:, :, 512:1024],
            in_=Wihi[:, 512:1024].rearrange("(k p) o -> p k o", p=P))
        nc.sync.dma_start(
            out=wi8[:, :, :, 512:1024],
            in_=Wi8[:, :, 512:1024].rearrange("c (k p) o -> p c k o", p=P))
        nc.sync.dma_start(
            out=wqf, in_=Wqf[:, :].rearrange("(k p) o -> p k o", p=P))
        nc.sync.dma_start(
            out=wcf, in_=Wcf[:, :].rearrange("(k p) o -> p k o", p=P))
        nc.sync.dma_start(out=bo_sb, in_=bo[:, :])
        nc.sync.dma_start(out=mask_sb,
                          in_=maskc[:, :, :].rearrange("b p m -> p b m"))
        ets[0] = [emit_et(0, m) for m in range(6)]
        qws[0] = step1(0)

        def step2(b):
            if b not in ets:
                ets[b] = [emit_et(b, m) for m in range(6)]
            for m in range(6, ST):
                ets[b].append(emit_et(b, m))
            if b + 1 < BPC:
                qs[b + 1] = load_q(b + 1)
            score = pcs.tile([P, ST, T], f32, tag="B", name=f"score_{b}")
            smax = sp.tile([P, T], f32, tag="smax")
            qwhi, qw8 = qws[b]
            for m in range(ST):
                eh, e8 = ets[b][m]
                mp = psB.tile([P, T], f32, tag="sc", name=f"scm_{b}_{m}")
                for k in range(HT):
                    nc.tensor.matmul(mp, eh[:, k, :], qwhi[:, k, :],
                                     start=(k == 0), stop=(k == HT - 1))
                cp = psB.tile([P, T], f32, tag="sc", name=f"scc_{b}_{m}")
                for k in range(HT):
                    nc.tensor.matmul(cp, e8[:, :, k, :], qw8[:, :, k, :],
                                     start=(k == 0), stop=(k == HT - 1),
                                     perf_mode=DR)
                nc.scalar.copy(score[:, m, :], mp)
                # fold cross back in-place: score = main + 2^-11 * cross
                nc.vector.scalar_tensor_tensor(score[:, m, :], cp, RSCI,
                                               score[:, m, :],
                                               ALU.mult, ALU.add)
                # max over UNMASKED positions only: with e stored in fp16,
                # a masked global max would flush every real exp below the
                # fp16 subnormal floor and zero the denominator.
                if m == 0:
                    nc.vector.tensor_scalar_add(smax, mp,
                                                mask_sb[:, b, m:m + 1])
                else:
                    nc.vector.scalar_tensor_tensor(smax, mp,
                                                   mask_sb[:, b, m:m + 1],
                                                   smax, ALU.add, ALU.max)
            return score, smax

        def softmax(b, score, smax):
            smax_all = sp.tile([P, T], f32, tag="smax_all")
            nc.gpsimd.partition_all_reduce(smax_all, smax, channels=P,
                                           reduce_op=bass_isa.ReduceOp.max)
            e = pe1.tile([P, ST, T], fp16, tag="E", name=f"e_{b}")
            for m in range(ST):
                nc.vector.tensor_sub(score[:, m, :], score[:, m, :], smax_all)
                nc.scalar.activation(e[:, m, :], score[:, m, :], AF.Exp,
                                     bias=mask_sb[:, b, m:m + 1])
            esum = sp.tile([P, T], f32, tag="smax")
            nc.vector.tensor_add(esum, e[:, 0, :], e[:, 1, :])
            for m in range(2, ST):
                nc.vector.tensor_add(esum, esum, e[:, m, :])
            esum_all = sp.tile([P, T], f32, tag="esum_all")
            nc.gpsimd.partition_all_reduce(esum_all, esum, channels=P,
                                           reduce_op=bass_isa.ReduceOp.add)
            rdenb = sp.tile([P, T], f32, tag="smax_all")
            nc.vector.reciprocal(rdenb, esum_all)
            return e, rdenb

        sm = {}
        for b in range(BPC):
            if b < BPC - 1:
                sc_b = step2(b)
                qws[b + 1] = step1(b + 1)
                sm[b] = softmax(b, *sc_b)
            e, rdenb = sm[b]

            # --- step 4: c~^T = enc @ e^T (fp16), fold in 1/denom ---
            cn = pcs.tile([P, HT, T], fp16, tag="B2", name=f"cn_{b}")
            for m in range(HT):
                en = enp.tile([P, ST, 128], fp16, tag="en", name=f"en_{b}_{m}")
                nc.sync.dma_start(
                    out=en,
                    in_=encf[b, :, 128 * m:128 * (m + 1)]
                    .rearrange("(k p) h -> p k h", p=P))
                c_ps = psC.tile([P, T], f32, tag="c", name=f"c_{b}_{m}")
                for k in range(ST):
                    nc.tensor.matmul(c_ps, en[:, k, :], e[:, k, :],
                                     start=(k == 0), stop=(k == ST - 1))
                nc.vector.tensor_mul(cn[:, m, :], c_ps, rdenb)

            if b == BPC - 2:
                # hoist the last batch's step2+softmax: its softmax chain
                # then hides under step5(b)'s PE window (no step1 remains
                # to cover it otherwise)
                sc_l = step2(BPC - 1)
                sm[BPC - 1] = softmax(BPC - 1, *sc_l)

            # --- step 5: out^T = tanh(WqT @ q^T + WcT @ cn + b), fp16 ---
            if b + 1 < BPC - 1:
                ets[b + 1] = [emit_et(b + 1, m) for m in range(6)]
            for m in range(HT):
                o_ps = psA.tile([P, T], f32, tag="qo", name=f"o_{b}_{m}")
                msl = slice(128 * m, 128 * (m + 1))
                for k in range(HT):
                    nc.tensor.matmul(o_ps, wqf[:, k, msl], qs[b][0][:, k, :],
                                     start=(k == 0), stop=False)
                for k in range(HT):
                    nc.tensor.matmul(o_ps, wcf[:, k, msl], cn[:, k, :],
                                     start=False, stop=(k == HT - 1))
                ot = otp.tile([P, T], f32, tag="ot")
                nc.scalar.activation(ot, o_ps, AF.Tanh, bias=bo_sb[:, m:m + 1])
                nc.scalar.dma_start(out=outT[b, 128 * m:128 * (m + 1), :],
                                    in_=ot)

    nc.compile()
    return nc


def _split16(x):
    """fp16 hi + fp8 pack [hi8, 2^11*lo in fp8] (moving order)."""
    hi = x.astype(np.float16)
    lo = (x - hi.astype(np.float32)) * RSC
    return hi, hi.astype(ml_dtypes.float8_e4m3), lo.astype(ml_dtypes.float8_e4m3)


def kernel(query, encoder_outputs, src_lengths, W_in, W_out, b_out):
    query = np.asarray(query, dtype=np.float32)
    encoder_outputs = np.ascontiguousarray(np.asarray(encoder_outputs, np.float32))
    src_lengths = np.asarray(src_lengths)
    W_in = np.asarray(W_in, dtype=np.float32)
    W_out = np.asarray(W_out, dtype=np.float32)
    b_out = np.asarray(b_out, dtype=np.float32)

    # --- shared (weight) inputs ---
    W_inT = np.ascontiguousarray(W_in.T)                    # [h, o]
    Wihi, Wih8, Wil8 = _split16(W_inT)
    Wi8 = np.ascontiguousarray(np.stack([Wil8, Wih8], axis=0))  # stat: [lo, hi]
    Wqf = np.ascontiguousarray(W_out[:, :H].T).astype(np.float16)
    Wcf = np.ascontiguousarray(W_out[:, H:].T).astype(np.float16)
    bo = np.ascontiguousarray(b_out.reshape(HT, P).T)       # [p, m]

    # --- per-core shards ---
    in_maps = []
    for c in range(NCORES):
        bs = slice(c * BPC, (c + 1) * BPC)
        q = query[bs]                                       # [BPC, T, H]
        encs = encoder_outputs[bs]                          # [BPC, S, H]
        lens = np.asarray(src_lengths[bs], dtype=np.int64)

        qTa = np.ascontiguousarray(q.transpose(0, 2, 1))    # [BPC, H, T]
        qh, qh8, ql8 = _split16(qTa)
        q8a = np.ascontiguousarray(np.stack([qh8, ql8], axis=1))  # mov: [hi, lo]
        eTa = np.ascontiguousarray(encs.transpose(0, 2, 1))  # [BPC, H, S]
        eh, eh8, el8 = _split16(eTa)
        eT8a = np.ascontiguousarray(np.stack([el8, eh8], axis=1))  # stat: [lo, hi]

        maskca = np.zeros((BPC, P, ST), dtype=np.float32)
        pos = (np.arange(ST)[None, :] * P + np.arange(P)[:, None])  # [P, ST]
        for j in range(BPC):
            maskca[j][pos >= lens[j]] = MASKVAL

        in_maps.append({
            "qhi": qh, "q8": q8a, "eThi": eh, "eT8": eT8a,
            "encf": encs.astype(np.float16),
            "maskc": maskca, "Wihi": Wihi, "Wi8": Wi8,
            "Wqf": Wqf, "Wcf": Wcf, "bo": bo,
        })

    if not _nc_cache:
        _nc_cache.append(_build_nc())
    nc = _nc_cache[0]

    res = run_bass_kernel_spmd(nc, in_maps, core_ids=list(range(NCORES)))
    global LAST_RESULTS
    LAST_RESULTS = res

    out = np.empty((B, T, H), dtype=np.float32)
    for c in range(NCORES):
        o = res.results[c]["outT"]                          # [BPC, H, T]
        out[c * BPC:(c + 1) * BPC] = o.transpose(0, 2, 1)
    return out


# revision 22
# speedup vs baseline: 1.0635x; 1.0036x over previous
"""Trainium2 Bass kernel for nn_Attention_12369505813001.

Computes, per batch b:
    qw    = query @ W_in.T                      [T, H]
    score = qw @ enc.T                          [T, S]
    p     = softmax(mask(score), axis=S)
    c     = p @ enc                             [T, H]
    out   = tanh(concat(query, c) @ W_out.T + b_out)

Shapes: B=32, T=512, S=1024, H=1024, fp32. Data-parallel over B across
8 NeuronCores (4 batches/core); no collectives.

Layout strategy (per core): feature dim on partitions, T on the free
axis throughout, so the PE contraction dim always lands on partitions
and no on-device transposes are needed:
    step1  qw^T[o,t]    = W_inT-tiles(stat) @ q^T(moving)
    step2  score^T[s,t] = encT-tiles(stat)  @ qw^T(moving)
    softmax over s (partition+chunk axis): per-batch global max via
      free-axis max tree + GPSIMD partition all-reduce(max); exp on ACT
      with per-partition bias = additive length mask; denominator via a
      DVE chunk-sum tree + GPSIMD partition all-reduce(add); the
      normalization is folded into c as a broadcast mul.
    step4  c~^T[h,t]    = enc-tiles(stat)   @ e^T(moving)     fp16
    step5  out^T[o,t]   = tanh(WqT(stat) @ q^T + WcT(stat) @ cn + b)  fp16

Precision: the softmax path must be accurate — score noise of 3e-3 rms
already breaches the 2e-2 gate at near-tie softmax columns — so steps
1-2 use a split scheme at 1.5 matmul passes: a main fp16 hi*hi pass
plus ONE fp8e4m3 DoubleRow pass computing both cross terms
(lo*hi + hi*lo) at 0.5 cyc/row. Residuals lo = x - fp16(x) are
prescaled by 2^11 into fp8 range; the cross PSUM is folded back as
score = main + 2^-11 * cross on DVE. Emulated end-to-end error 3.9e-3.
Splits of q, encT, W_in are host-precomputed; qw is split on device.
Steps 4-5 are insensitive and run fully in fp16. Because e is stored
fp16 (subnormal floor ~6e-8), the softmax max MUST exclude masked
positions — a masked global max would flush every real exp to zero and
divide by a zero denominator — so the mask is folded into the max tree
(fused add+max against the per-partition mask scalar).

Schedule: software-pipelined across batches — step1(b+1) is emitted
between step2(b) and softmax(b), so the PE stays busy through the
softmax chain (which runs on DVE/ACT/GPSIMD). DMA is dominated by a
serial descriptor-generation path, so the kernel uses few, large DMA
instructions, emitted in the exact order they are consumed, with et
(encT) tiles prefetched one phase early. Output DMAs issue from the
Activation queue so their tanh-producer waits never block the input
DMA stream on SP.
"""

from contextlib import ExitStack

import numpy as np
import ml_dtypes

import concourse.bass as bass
import concourse.bass_isa as bass_isa
import concourse.mybir as mybir
import concourse.tile as tile
from concourse import bacc
from concourse.bass_utils import run_bass_kernel_spmd

B, T, S, H = 32, 512, 1024, 1024
NCORES = 8
BPC = B // NCORES          # batches per core
HT = H // 128              # h/o chunk count
ST = S // 128              # s chunk count
P = 128

f32 = mybir.dt.float32
f32r = mybir.dt.float32r
bf16 = mybir.dt.bfloat16
fp16 = mybir.dt.float16
fp8 = mybir.dt.float8e4
AF = mybir.ActivationFunctionType
ALU = mybir.AluOpType
DR = mybir.MatmulPerfMode.DoubleRow

MASKVAL = -1.0e38
RSC = 2048.0               # 2^11 residual prescale
RSCI = 1.0 / RSC

_nc_cache = []
LAST_RESULTS = None


def _build_nc():
    nc = bacc.Bacc("TRN2", target_bir_lowering=False, debug=False)

    # moving packs: [hi8, lo8'] pairs; stationary packs: [lo8', hi8]
    qhi = nc.dram_tensor("qhi", [BPC, H, T], fp16, kind="ExternalInput")
    q8 = nc.dram_tensor("q8", [BPC, 2, H, T], fp8, kind="ExternalInput")
    eThi = nc.dram_tensor("eThi", [BPC, H, S], fp16, kind="ExternalInput")
    eT8 = nc.dram_tensor("eT8", [BPC, 2, H, S], fp8, kind="ExternalInput")
    encf = nc.dram_tensor("encf", [BPC, S, H], fp16, kind="ExternalInput")
    maskc = nc.dram_tensor("maskc", [BPC, P, ST], f32, kind="ExternalInput")
    Wihi = nc.dram_tensor("Wihi", [H, H], fp16, kind="ExternalInput")  # [h,o]
    Wi8 = nc.dram_tensor("Wi8", [2, H, H], fp8, kind="ExternalInput")
    Wqf = nc.dram_tensor("Wqf", [H, H], fp16, kind="ExternalInput")
    Wcf = nc.dram_tensor("Wcf", [H, H], fp16, kind="ExternalInput")
    bo = nc.dram_tensor("bo", [P, HT], f32, kind="ExternalInput")
    outT = nc.dram_tensor("outT", [BPC, H, T], f32, kind="ExternalOutput")

    with tile.TileContext(nc) as tc, ExitStack() as ctx:
        wp = ctx.enter_context(tc.tile_pool(name="wp", bufs=1))
        pq = ctx.enter_context(tc.tile_pool(name="pq", bufs=2))
        pa = ctx.enter_context(tc.tile_pool(name="pa", bufs=2))   # qw / e
        pcs = ctx.enter_context(tc.tile_pool(name="pcs", bufs=1))  # score/cn
        tp = ctx.enter_context(tc.tile_pool(name="tp", bufs=1))
        pe1 = ctx.enter_context(tc.tile_pool(name="pe1", bufs=1))
        sp = ctx.enter_context(tc.tile_pool(name="sp", bufs=1))
        etp = ctx.enter_context(tc.tile_pool(name="etp", bufs=6))
        et8p = ctx.enter_context(tc.tile_pool(name="et8p", bufs=1))
        enp = ctx.enter_context(tc.tile_pool(name="enp", bufs=3))
        otp = ctx.enter_context(tc.tile_pool(name="otp", bufs=2))
        psA = ctx.enter_context(tc.tile_pool(name="psA", bufs=2, space="PSUM"))
        psB = ctx.enter_context(tc.tile_pool(name="psB", bufs=2, space="PSUM"))
        psC = ctx.enter_context(tc.tile_pool(name="psC", bufs=2, space="PSUM"))

        # --- persistent weights (whole-tensor DMAs; o-contiguous runs) ---
        wihi = wp.tile([P, HT, H], fp16, name="wihi")
        wi8 = wp.tile([P, 2, HT, H], fp8, name="wi8")
        wqf = wp.tile([P, HT, H], fp16, name="wqf")
        wcf = wp.tile([P, HT, H], fp16, name="wcf")
        bo_sb = wp.tile([P, HT], f32)
        mask_sb = wp.tile([P, BPC, ST], f32)

        def load_q(b):
            th = pq.tile([P, HT, T], fp16, tag="qhi", name=f"qhi_{b}")
            nc.sync.dma_start(
                out=th, in_=qhi[b].rearrange("(k p) t -> p k t", p=P))
            t8 = pq.tile([P, 2, HT, T], fp8, tag="q8", name=f"q8_{b}")
            nc.sync.dma_start(
                out=t8, in_=q8[b].rearrange("c (k p) t -> p c k t", p=P))
            return th, t8

        def emit_et(b, m):
            eh = etp.tile([P, HT, 128], fp16, tag="et", name=f"et_{b}_{m}")
            nc.sync.dma_start(
                out=eh,
                in_=eThi[b, :, 128 * m:128 * (m + 1)]
                .rearrange("(k p) s -> p k s", p=P))
            return eh

        e8ws = {}

        def load_et8w(b):
            # whole-batch fp8 cross pack: 1KB contiguous runs avoid the
            # sub-512B descriptor latency penalty of per-m tiles
            t = et8p.tile([P, 2, HT, S], fp8, tag="e8w", name=f"e8w_{b}")
            nc.sync.dma_start(
                out=t, in_=eT8[b].rearrange("c (k p) s -> p c k s", p=P))
            return t

        qs = {}
        qws = {}
        ets = {}

        def step1(b):
            qwhi = pa.tile([P, HT, T], fp16, tag="A", name=f"qwhi_{b}")
            qw8 = pa.tile([P, 2, HT, T], fp8, tag="A8", name=f"qw8_{b}")
            th, t8 = qs[b]
            for m in range(HT):
                msl = slice(128 * m, 128 * (m + 1))
                mp = psA.tile([P, T], f32, tag="qo", name=f"qwm_{b}_{m}")
                for k in range(HT):
                    nc.tensor.matmul(mp, wihi[:, k, msl], th[:, k, :],
                                     start=(k == 0), stop=(k == HT - 1))
                cp = psA.tile([P, T], f32, tag="qo", name=f"qwc_{b}_{m}")
                for k in range(HT):
                    nc.tensor.matmul(cp, wi8[:, :, k, msl], t8[:, :, k, :],
                                     start=(k == 0), stop=(k == HT - 1),
                                     perf_mode=DR)
                tmp = tp.tile([P, T], f32, tag="tmp")
                nc.vector.tensor_scalar_mul(tmp, cp, RSCI)
                nc.vector.tensor_add(tmp, tmp, mp)
                nc.scalar.copy(qwhi[:, m, :], tmp)
                nc.scalar.copy(qw8[:, 0, m, :], qwhi[:, m, :])
                nc.vector.tensor_sub(tmp, tmp, qwhi[:, m, :])
                nc.scalar.activation(qw8[:, 1, m, :], tmp, AF.Copy, scale=RSC)
            return qwhi, qw8

        # --- prologue ---
        # weight slices interleaved with q so step1(0) passes start early
        nc.sync.dma_start(
            out=wihi[:, :, 0:128],
            in_=Wihi[:, 0:128].rearrange("(k p) o -> p k o", p=P))
        qs[0] = load_q(0)
        nc.sync.dma_start(
            out=wi8[:, :, :, 0:128],
            in_=Wi8[:, :, 0:128].rearrange("c (k p) o -> p c k o", p=P))
        nc.sync.dma_start(
            out=wihi[:, :, 128:512],
            in_=Wihi[:, 128:512].rearrange("(k p) o -> p k o", p=P))
        nc.sync.dma_start(
            out=wi8[:, :, :, 128:512],
            in_=Wi8[:, :, 128:512].rearrange("c (k p) o -> p c k o", p=P))
        nc.sync.dma_start(
            out=wihi[:, :, 512:1024],
            in_=Wihi[:, 512:1024].rearrange("(k p) o -> p k o", p=P))
        nc.sync.dma_start(
            out=wi8[:, :, :, 512:1024],
            in_=Wi8[:, :, 512:1024].rearrange("c (k p) o -> p c k o", p=P))
        nc.sync.dma_start(
            out=wqf, in_=Wqf[:, :].rearrange("(k p) o -> p k o", p=P))
        nc.sync.dma_start(
            out=wcf, in_=Wcf[:, :].rearrange("(k p) o -> p k o", p=P))
        nc.sync.dma_start(out=bo_sb, in_=bo[:, :])
        nc.sync.dma_start(out=mask_sb,
                          in_=maskc[:, :, :].rearrange("b p m -> p b m"))
        e8ws[0] = load_et8w(0)
        ets[0] = [emit_et(0, m) for m in range(6)]
        qws[0] = step1(0)

        def step2(b):
            if b not in ets:
                e8ws[b] = load_et8w(b)
                ets[b] = [emit_et(b, m) for m in range(6)]
            for m in range(6, ST):
                ets[b].append(emit_et(b, m))
            if b + 1 < BPC:
                qs[b + 1] = load_q(b + 1)
            score = pcs.tile([P, ST, T], f32, tag="B", name=f"score_{b}")
            smax = sp.tile([P, T], f32, tag="smax")
            qwhi, qw8 = qws[b]
            e8w = e8ws[b]
            for m in range(ST):
                eh = ets[b][m]
                msl2 = slice(128 * m, 128 * (m + 1))
                mp = psB.tile([P, T], f32, tag="sc", name=f"scm_{b}_{m}")
                for k in range(HT):
                    nc.tensor.matmul(mp, eh[:, k, :], qwhi[:, k, :],
                                     start=(k == 0), stop=(k == HT - 1))
                cp = psB.tile([P, T], f32, tag="sc", name=f"scc_{b}_{m}")
                for k in range(HT):
                    nc.tensor.matmul(cp, e8w[:, :, k, msl2], qw8[:, :, k, :],
                                     start=(k == 0), stop=(k == HT - 1),
                                     perf_mode=DR)
                nc.scalar.copy(score[:, m, :], mp)
                # fold cross back in-place: score = main + 2^-11 * cross
                nc.vector.scalar_tensor_tensor(score[:, m, :], cp, RSCI,
                                               score[:, m, :],
                                               ALU.mult, ALU.add)
                # max over UNMASKED positions only: with e stored in fp16,
                # a masked global max would flush every real exp below the
                # fp16 subnormal floor and zero the denominator.
                if m == 0:
                    nc.vector.tensor_scalar_add(smax, mp,
                                                mask_sb[:, b, m:m + 1])
                else:
                    nc.vector.scalar_tensor_tensor(smax, mp,
                                                   mask_sb[:, b, m:m + 1],
                                                   smax, ALU.add, ALU.max)
            return score, smax

        def softmax(b, score, smax):
            smax_all = sp.tile([P, T], f32, tag="smax_all")
            nc.gpsimd.partition_all_reduce(smax_all, smax, channels=P,
                                           reduce_op=bass_isa.ReduceOp.max)
            e = pe1.tile([P, ST, T], fp16, tag="E", name=f"e_{b}")
            for m in range(ST):
                nc.vector.tensor_sub(score[:, m, :], score[:, m, :], smax_all)
                nc.scalar.activation(e[:, m, :], score[:, m, :], AF.Exp,
                                     bias=mask_sb[:, b, m:m + 1])
            esum = sp.tile([P, T], f32, tag="smax")
            nc.vector.tensor_add(esum, e[:, 0, :], e[:, 1, :])
            for m in range(2, ST):
                nc.vector.tensor_add(esum, esum, e[:, m, :])
            esum_all = sp.tile([P, T], f32, tag="esum_all")
            nc.gpsimd.partition_all_reduce(esum_all, esum, channels=P,
                                           reduce_op=bass_isa.ReduceOp.add)
            rdenb = sp.tile([P, T], f32, tag="smax_all")
            nc.vector.reciprocal(rdenb, esum_all)
            if b == BPC - 2:
                # prefetch the hoisted last batch's encT tiles here, ahead
                # of step4(b)'s enc queue
                e8ws[BPC - 1] = load_et8w(BPC - 1)
                ets[BPC - 1] = [emit_et(BPC - 1, m) for m in range(6)]
            return e, rdenb

        sm = {}
        for b in range(BPC):
            if b < BPC - 1:
                sc_b = step2(b)
                qws[b + 1] = step1(b + 1)
                sm[b] = softmax(b, *sc_b)
            e, rdenb = sm[b]

            # --- step 4: c~^T = enc @ e^T (fp16), fold in 1/denom ---
            cn = pcs.tile([P, HT, T], fp16, tag="B2", name=f"cn_{b}")
            for m in range(HT):
                en = enp.tile([P, ST, 128], fp16, tag="en", name=f"en_{b}_{m}")
                nc.sync.dma_start(
                    out=en,
                    in_=encf[b, :, 128 * m:128 * (m + 1)]
                    .rearrange("(k p) h -> p k h", p=P))
                c_ps = psC.tile([P, T], f32, tag="c", name=f"c_{b}_{m}")
                for k in range(ST):
                    nc.tensor.matmul(c_ps, en[:, k, :], e[:, k, :],
                                     start=(k == 0), stop=(k == ST - 1))
                nc.vector.tensor_mul(cn[:, m, :], c_ps, rdenb)

            if b == BPC - 2:
                # hoist the last batch's step2+softmax: its softmax chain
                # then hides under step5(b)'s PE window (no step1 remains
                # to cover it otherwise)
                sc_l = step2(BPC - 1)
                sm[BPC - 1] = softmax(BPC - 1, *sc_l)

            # --- step 5: out^T = tanh(WqT @ q^T + WcT @ cn + b), fp16 ---
            if b + 1 < BPC - 1:
                e8ws[b + 1] = load_et8w(b + 1)
                ets[b + 1] = [emit_et(b + 1, m) for m in range(6)]
            for m in range(HT):
                o_ps = psA.tile([P, T], f32, tag="qo", name=f"o_{b}_{m}")
                msl = slice(128 * m, 128 * (m + 1))
                for k in range(HT):
                    nc.tensor.matmul(o_ps, wqf[:, k, msl], qs[b][0][:, k, :],
                                     start=(k == 0), stop=False)
                for k in range(HT):
                    nc.tensor.matmul(o_ps, wcf[:, k, msl], cn[:, k, :],
                                     start=False, stop=(k == HT - 1))
                ot = otp.tile([P, T], f32, tag="ot")
                nc.scalar.activation(ot, o_ps, AF.Tanh, bias=bo_sb[:, m:m + 1])
                nc.scalar.dma_start(out=outT[b, 128 * m:128 * (m + 1), :],
                                    in_=ot)

    nc.compile()
    return nc


def _split16(x):
    """fp16 hi + fp8 pack [hi8, 2^11*lo in fp8] (moving order)."""
    hi = x.astype(np.float16)
    lo = (x - hi.astype(np.float32)) * RSC
    return hi, hi.astype(ml_dtypes.float8_e4m3), lo.astype(ml_dtypes.float8_e4m3)


def kernel(query, encoder_outputs, src_lengths, W_in, W_out, b_out):
    query = np.asarray(query, dtype=np.float32)
    encoder_outputs = np.ascontiguousarray(np.asarray(encoder_outputs, np.float32))
    src_lengths = np.asarray(src_lengths)
    W_in = np.asarray(W_in, dtype=np.float32)
    W_out = np.asarray(W_out, dtype=np.float32)
    b_out = np.asarray(b_out, dtype=np.float32)

    # --- shared (weight) inputs ---
    W_inT = np.ascontiguousarray(W_in.T)                    # [h, o]
    Wihi, Wih8, Wil8 = _split16(W_inT)
    Wi8 = np.ascontiguousarray(np.stack([Wil8, Wih8], axis=0))  # stat: [lo, hi]
    Wqf = np.ascontiguousarray(W_out[:, :H].T).astype(np.float16)
    Wcf = np.ascontiguousarray(W_out[:, H:].T).astype(np.float16)
    bo = np.ascontiguousarray(b_out.reshape(HT, P).T)       # [p, m]

    # --- per-core shards ---
    in_maps = []
    for c in range(NCORES):
        bs = slice(c * BPC, (c + 1) * BPC)
        q = query[bs]                                       # [BPC, T, H]
        encs = encoder_outputs[bs]                          # [BPC, S, H]
        lens = np.asarray(src_lengths[bs], dtype=np.int64)

        qTa = np.ascontiguousarray(q.transpose(0, 2, 1))    # [BPC, H, T]
        qh, qh8, ql8 = _split16(qTa)
        q8a = np.ascontiguousarray(np.stack([qh8, ql8], axis=1))  # mov: [hi, lo]
        eTa = np.ascontiguousarray(encs.transpose(0, 2, 1))  # [BPC, H, S]
        eh, eh8, el8 = _split16(eTa)
        eT8a = np.ascontiguousarray(np.stack([el8, eh8], axis=1))  # stat: [lo, hi]

        maskca = np.zeros((BPC, P, ST), dtype=np.float32)
        pos = (np.arange(ST)[None, :] * P + np.arange(P)[:, None])  # [P, ST]
        for j in range(BPC):
            maskca[j][pos >= lens[j]] = MASKVAL

        in_maps.append({
            "qhi": qh, "q8": q8a, "eThi": eh, "eT8": eT8a,
            "encf": encs.astype(np.float16),
            "maskc": maskca, "Wihi": Wihi, "Wi8": Wi8,
            "Wqf": Wqf, "Wcf": Wcf, "bo": bo,
        })

    if not _nc_cache:
        _nc_cache.append(_build_nc())
    nc = _nc_cache[0]

    res = run_bass_kernel_spmd(nc, in_maps, core_ids=list(range(NCORES)))
    global LAST_RESULTS
    LAST_RESULTS = res

    out = np.empty((B, T, H), dtype=np.float32)
    for c in range(NCORES):
        o = res.results[c]["outT"]                          # [BPC, H, T]
        out[c * BPC:(c + 1) * BPC] = o.transpose(0, 2, 1)
    return out


# revision 23
# speedup vs baseline: 1.0851x; 1.0203x over previous
"""Trainium2 Bass kernel for nn_Attention_12369505813001.

Computes, per batch b:
    qw    = query @ W_in.T                      [T, H]
    score = qw @ enc.T                          [T, S]
    p     = softmax(mask(score), axis=S)
    c     = p @ enc                             [T, H]
    out   = tanh(concat(query, c) @ W_out.T + b_out)

Shapes: B=32, T=512, S=1024, H=1024, fp32. Data-parallel over B across
8 NeuronCores (4 batches/core); no collectives.

Layout strategy (per core): feature dim on partitions, T on the free
axis throughout, so the PE contraction dim always lands on partitions
and no on-device transposes are needed:
    step1  qw^T[o,t]    = W_inT-tiles(stat) @ q^T(moving)
    step2  score^T[s,t] = encT-tiles(stat)  @ qw^T(moving)
    softmax over s (partition+chunk axis): per-batch global max via
      free-axis max tree + GPSIMD partition all-reduce(max); exp on ACT
      with per-partition bias = additive length mask; denominator via a
      DVE chunk-sum tree + GPSIMD partition all-reduce(add); the
      normalization is folded into c as a broadcast mul.
    step4  c~^T[h,t]    = enc-tiles(stat)   @ e^T(moving)     fp16
    step5  out^T[o,t]   = tanh(WqT(stat) @ q^T + WcT(stat) @ cn + b)  fp16

Precision: the softmax path must be accurate — score noise of 3e-3 rms
already breaches the 2e-2 gate at near-tie softmax columns — so steps
1-2 use a split scheme at 1.5 matmul passes: a main fp16 hi*hi pass
plus ONE fp8e4m3 DoubleRow pass computing both cross terms
(lo*hi + hi*lo) at 0.5 cyc/row. Residuals lo = x - fp16(x) are
prescaled by 2^11 into fp8 range; the cross PSUM is folded back as
score = main + 2^-11 * cross on DVE. Emulated end-to-end error 3.9e-3.
Splits of q, encT, W_in are host-precomputed; qw is split on device.
Steps 4-5 are insensitive and run fully in fp16. Because e is stored
fp16 (subnormal floor ~6e-8), the softmax max MUST exclude masked
positions — a masked global max would flush every real exp to zero and
divide by a zero denominator — so the mask is folded into the max tree
(fused add+max against the per-partition mask scalar).

Schedule: software-pipelined across batches — step1(b+1) is emitted
between step2(b) and softmax(b), so the PE stays busy through the
softmax chain (which runs on DVE/ACT/GPSIMD). DMA is dominated by a
serial descriptor-generation path, so the kernel uses few, large DMA
instructions, emitted in the exact order they are consumed, with et
(encT) tiles prefetched one phase early. Output DMAs issue from the
Activation queue so their tanh-producer waits never block the input
DMA stream on SP.
"""

from contextlib import ExitStack

import numpy as np
import ml_dtypes

import concourse.bass as bass
import concourse.bass_isa as bass_isa
import concourse.mybir as mybir
import concourse.tile as tile
from concourse import bacc
from concourse.bass_utils import run_bass_kernel_spmd

B, T, S, H = 32, 512, 1024, 1024
NCORES = 8
BPC = B // NCORES          # batches per core
HT = H // 128              # h/o chunk count
ST = S // 128              # s chunk count
P = 128

f32 = mybir.dt.float32
f32r = mybir.dt.float32r
bf16 = mybir.dt.bfloat16
fp16 = mybir.dt.float16
fp8 = mybir.dt.float8e4
AF = mybir.ActivationFunctionType
ALU = mybir.AluOpType
DR = mybir.MatmulPerfMode.DoubleRow

MASKVAL = -1.0e38
RSC = 2048.0               # 2^11 residual prescale
RSCI = 1.0 / RSC

_nc_cache = []
LAST_RESULTS = None


def _build_nc():
    nc = bacc.Bacc("TRN2", target_bir_lowering=False, debug=False)

    # moving packs: [hi8, lo8'] pairs; stationary packs: [lo8', hi8]
    qhi = nc.dram_tensor("qhi", [BPC, H, T], fp16, kind="ExternalInput")
    q8 = nc.dram_tensor("q8", [BPC, 2, H, T], fp8, kind="ExternalInput")
    eThi = nc.dram_tensor("eThi", [BPC, H, S], fp16, kind="ExternalInput")
    eT8 = nc.dram_tensor("eT8", [BPC, 2, H, S], fp8, kind="ExternalInput")
    encf = nc.dram_tensor("encf", [BPC, S, H], fp16, kind="ExternalInput")
    maskc = nc.dram_tensor("maskc", [BPC, P, ST], f32, kind="ExternalInput")
    Wihi = nc.dram_tensor("Wihi", [H, H], fp16, kind="ExternalInput")  # [h,o]
    Wi8 = nc.dram_tensor("Wi8", [2, H, H], fp8, kind="ExternalInput")
    Wqf = nc.dram_tensor("Wqf", [H, H], fp16, kind="ExternalInput")
    Wcf = nc.dram_tensor("Wcf", [H, H], fp16, kind="ExternalInput")
    bo = nc.dram_tensor("bo", [P, HT], f32, kind="ExternalInput")
    outT = nc.dram_tensor("outT", [BPC, H, T], f32, kind="ExternalOutput")

    with tile.TileContext(nc) as tc, ExitStack() as ctx:
        wp = ctx.enter_context(tc.tile_pool(name="wp", bufs=1))
        pq = ctx.enter_context(tc.tile_pool(name="pq", bufs=2))
        pa = ctx.enter_context(tc.tile_pool(name="pa", bufs=2))   # qw / e
        pcs = ctx.enter_context(tc.tile_pool(name="pcs", bufs=1))  # score/cn
        tp = ctx.enter_context(tc.tile_pool(name="tp", bufs=1))
        pe1 = ctx.enter_context(tc.tile_pool(name="pe1", bufs=1))
        sp = ctx.enter_context(tc.tile_pool(name="sp", bufs=1))
        etp = ctx.enter_context(tc.tile_pool(name="etp", bufs=6))
        et8p = ctx.enter_context(tc.tile_pool(name="et8p", bufs=1))
        enp = ctx.enter_context(tc.tile_pool(name="enp", bufs=3))
        otp = ctx.enter_context(tc.tile_pool(name="otp", bufs=2))
        psA = ctx.enter_context(tc.tile_pool(name="psA", bufs=4, space="PSUM"))
        psB = ctx.enter_context(tc.tile_pool(name="psB", bufs=2, space="PSUM"))
        psC = ctx.enter_context(tc.tile_pool(name="psC", bufs=2, space="PSUM"))

        # --- persistent weights (whole-tensor DMAs; o-contiguous runs) ---
        wihi = wp.tile([P, HT, H], fp16, name="wihi")
        wi8 = wp.tile([P, 2, HT, H], fp8, name="wi8")
        wqf = wp.tile([P, HT, H], fp16, name="wqf")
        wcf = wp.tile([P, HT, H], fp16, name="wcf")
        bo_sb = wp.tile([P, HT], f32)
        mask_sb = wp.tile([P, BPC, ST], f32)

        def load_q(b):
            th = pq.tile([P, HT, T], fp16, tag="qhi", name=f"qhi_{b}")
            nc.sync.dma_start(
                out=th, in_=qhi[b].rearrange("(k p) t -> p k t", p=P))
            t8 = pq.tile([P, 2, HT, T], fp8, tag="q8", name=f"q8_{b}")
            nc.sync.dma_start(
                out=t8, in_=q8[b].rearrange("c (k p) t -> p c k t", p=P))
            return th, t8

        def emit_et(b, m):
            eh = etp.tile([P, HT, 128], fp16, tag="et", name=f"et_{b}_{m}")
            nc.sync.dma_start(
                out=eh,
                in_=eThi[b, :, 128 * m:128 * (m + 1)]
                .rearrange("(k p) s -> p k s", p=P))
            return eh

        e8ws = {}

        def load_et8w(b):
            # whole-batch fp8 cross pack: 1KB contiguous runs avoid the
            # sub-512B descriptor latency penalty of per-m tiles
            t = et8p.tile([P, 2, HT, S], fp8, tag="e8w", name=f"e8w_{b}")
            nc.sync.dma_start(
                out=t, in_=eT8[b].rearrange("c (k p) s -> p c k s", p=P))
            return t

        qs = {}
        qws = {}
        ets = {}

        def step1(b):
            qwhi = pa.tile([P, HT, T], fp16, tag="A", name=f"qwhi_{b}")
            qw8 = pa.tile([P, 2, HT, T], fp8, tag="A8", name=f"qw8_{b}")
            th, t8 = qs[b]

            def emit_main(m):
                msl = slice(128 * m, 128 * (m + 1))
                mp = psA.tile([P, T], f32, tag="qo", name=f"qwm_{b}_{m}")
                for k in range(HT):
                    nc.tensor.matmul(mp, wihi[:, k, msl], th[:, k, :],
                                     start=(k == 0), stop=(k == HT - 1))
                return mp

            def emit_cross_chain(m, mp):
                msl = slice(128 * m, 128 * (m + 1))
                cp = psA.tile([P, T], f32, tag="qo", name=f"qwc_{b}_{m}")
                for k in range(HT):
                    nc.tensor.matmul(cp, wi8[:, :, k, msl], t8[:, :, k, :],
                                     start=(k == 0), stop=(k == HT - 1),
                                     perf_mode=DR)
                tmp = tp.tile([P, T], f32, tag="tmp")
                nc.vector.tensor_scalar_mul(tmp, cp, RSCI)
                nc.vector.tensor_add(tmp, tmp, mp)
                nc.scalar.copy(qwhi[:, m, :], tmp)
                nc.scalar.copy(qw8[:, 0, m, :], qwhi[:, m, :])
                nc.vector.tensor_sub(tmp, tmp, qwhi[:, m, :])
                nc.scalar.activation(qw8[:, 1, m, :], tmp, AF.Copy, scale=RSC)

            if b == 0:
                # head: run two fp16 main chunks first so the PE has work
                # while the fp8 cross operands (wi8/q8) are still arriving
                mp0 = emit_main(0)
                mp1 = emit_main(1)
                emit_cross_chain(0, mp0)
                emit_cross_chain(1, mp1)
                rest = range(2, HT)
            else:
                rest = range(HT)
            for m in rest:
                emit_cross_chain(m, emit_main(m))
            return qwhi, qw8

        # --- prologue ---
        # weight slices interleaved with q so step1(0) passes start early
        nc.sync.dma_start(
            out=wihi[:, :, 0:128],
            in_=Wihi[:, 0:128].rearrange("(k p) o -> p k o", p=P))
        qs[0] = load_q(0)
        nc.sync.dma_start(
            out=wi8[:, :, :, 0:128],
            in_=Wi8[:, :, 0:128].rearrange("c (k p) o -> p c k o", p=P))
        nc.sync.dma_start(
            out=wihi[:, :, 128:512],
            in_=Wihi[:, 128:512].rearrange("(k p) o -> p k o", p=P))
        nc.sync.dma_start(
            out=wi8[:, :, :, 128:512],
            in_=Wi8[:, :, 128:512].rearrange("c (k p) o -> p c k o", p=P))
        nc.sync.dma_start(
            out=wihi[:, :, 512:1024],
            in_=Wihi[:, 512:1024].rearrange("(k p) o -> p k o", p=P))
        nc.sync.dma_start(
            out=wi8[:, :, :, 512:1024],
            in_=Wi8[:, :, 512:1024].rearrange("c (k p) o -> p c k o", p=P))
        nc.sync.dma_start(
            out=wqf, in_=Wqf[:, :].rearrange("(k p) o -> p k o", p=P))
        nc.sync.dma_start(
            out=wcf, in_=Wcf[:, :].rearrange("(k p) o -> p k o", p=P))
        nc.sync.dma_start(out=bo_sb, in_=bo[:, :])
        nc.sync.dma_start(out=mask_sb,
                          in_=maskc[:, :, :].rearrange("b p m -> p b m"))
        e8ws[0] = load_et8w(0)
        ets[0] = [emit_et(0, m) for m in range(6)]
        qws[0] = step1(0)

        def step2(b):
            if b not in ets:
                e8ws[b] = load_et8w(b)
                ets[b] = [emit_et(b, m) for m in range(6)]
            for m in range(6, ST):
                ets[b].append(emit_et(b, m))
            if b + 1 < BPC:
                qs[b + 1] = load_q(b + 1)
            score = pcs.tile([P, ST, T], f32, tag="B", name=f"score_{b}")
            smax = sp.tile([P, T], f32, tag="smax")
            qwhi, qw8 = qws[b]
            e8w = e8ws[b]
            for m in range(ST):
                eh = ets[b][m]
                msl2 = slice(128 * m, 128 * (m + 1))
                mp = psB.tile([P, T], f32, tag="sc", name=f"scm_{b}_{m}")
                for k in range(HT):
                    nc.tensor.matmul(mp, eh[:, k, :], qwhi[:, k, :],
                                     start=(k == 0), stop=(k == HT - 1))
                cp = psB.tile([P, T], f32, tag="sc", name=f"scc_{b}_{m}")
                for k in range(HT):
                    nc.tensor.matmul(cp, e8w[:, :, k, msl2], qw8[:, :, k, :],
                                     start=(k == 0), stop=(k == HT - 1),
                                     perf_mode=DR)
                nc.scalar.copy(score[:, m, :], mp)
                # fold cross back in-place: score = main + 2^-11 * cross
                nc.vector.scalar_tensor_tensor(score[:, m, :], cp, RSCI,
                                               score[:, m, :],
                                               ALU.mult, ALU.add)
                # max over UNMASKED positions only: with e stored in fp16,
                # a masked global max would flush every real exp below the
                # fp16 subnormal floor and zero the denominator.
                if m == 0:
                    nc.vector.tensor_scalar_add(smax, mp,
                                                mask_sb[:, b, m:m + 1])
                else:
                    nc.vector.scalar_tensor_tensor(smax, mp,
                                                   mask_sb[:, b, m:m + 1],
                                                   smax, ALU.add, ALU.max)
            return score, smax

        def softmax(b, score, smax):
            smax_all = sp.tile([P, T], f32, tag="smax_all")
            nc.gpsimd.partition_all_reduce(smax_all, smax, channels=P,
                                           reduce_op=bass_isa.ReduceOp.max)
            e = pe1.tile([P, ST, T], fp16, tag="E", name=f"e_{b}")
            for m in range(ST):
                nc.vector.tensor_sub(score[:, m, :], score[:, m, :], smax_all)
                nc.scalar.activation(e[:, m, :], score[:, m, :], AF.Exp,
                                     bias=mask_sb[:, b, m:m + 1])
            esum = sp.tile([P, T], f32, tag="smax")
            nc.vector.tensor_add(esum, e[:, 0, :], e[:, 1, :])
            for m in range(2, ST):
                nc.vector.tensor_add(esum, esum, e[:, m, :])
            esum_all = sp.tile([P, T], f32, tag="esum_all")
            nc.gpsimd.partition_all_reduce(esum_all, esum, channels=P,
                                           reduce_op=bass_isa.ReduceOp.add)
            rdenb = sp.tile([P, T], f32, tag="smax_all")
            nc.vector.reciprocal(rdenb, esum_all)
            if b == BPC - 2:
                # prefetch the hoisted last batch's encT tiles here, ahead
                # of step4(b)'s enc queue
                e8ws[BPC - 1] = load_et8w(BPC - 1)
                ets[BPC - 1] = [emit_et(BPC - 1, m) for m in range(6)]
            return e, rdenb

        sm = {}
        for b in range(BPC):
            if b < BPC - 1:
                sc_b = step2(b)
                qws[b + 1] = step1(b + 1)
                sm[b] = softmax(b, *sc_b)
            e, rdenb = sm[b]

            # --- step 4: c~^T = enc @ e^T (fp16), fold in 1/denom ---
            cn = pcs.tile([P, HT, T], fp16, tag="B2", name=f"cn_{b}")
            for m in range(HT):
                en = enp.tile([P, ST, 128], fp16, tag="en", name=f"en_{b}_{m}")
                nc.sync.dma_start(
                    out=en,
                    in_=encf[b, :, 128 * m:128 * (m + 1)]
                    .rearrange("(k p) h -> p k h", p=P))
                c_ps = psC.tile([P, T], f32, tag="c", name=f"c_{b}_{m}")
                for k in range(ST):
                    nc.tensor.matmul(c_ps, en[:, k, :], e[:, k, :],
                                     start=(k == 0), stop=(k == ST - 1))
                nc.vector.tensor_mul(cn[:, m, :], c_ps, rdenb)

            if b == BPC - 2:
                # hoist the last batch's step2+softmax: its softmax chain
                # then hides under step5(b)'s PE window (no step1 remains
                # to cover it otherwise)
                sc_l = step2(BPC - 1)
                sm[BPC - 1] = softmax(BPC - 1, *sc_l)

            # --- step 5: out^T = tanh(WqT @ q^T + WcT @ cn + b), fp16 ---
            if b + 1 < BPC - 1:
                e8ws[b + 1] = load_et8w(b + 1)
                ets[b + 1] = [emit_et(b + 1, m) for m in range(6)]
            for m in range(HT):
                o_ps = psA.tile([P, T], f32, tag="qo", name=f"o_{b}_{m}")
                msl = slice(128 * m, 128 * (m + 1))
                for k in range(HT):
                    nc.tensor.matmul(o_ps, wqf[:, k, msl], qs[b][0][:, k, :],
                                     start=(k == 0), stop=False)
                for k in range(HT):
                    nc.tensor.matmul(o_ps, wcf[:, k, msl], cn[:, k, :],
                                     start=False, stop=(k == HT - 1))
                ot = otp.tile([P, T], f32, tag="ot")
                nc.scalar.activation(ot, o_ps, AF.Tanh, bias=bo_sb[:, m:m + 1])
                nc.scalar.dma_start(out=outT[b, 128 * m:128 * (m + 1), :],
                                    in_=ot)

    nc.compile()
    return nc


def _split16(x):
    """fp16 hi + fp8 pack [hi8, 2^11*lo in fp8] (moving order)."""
    hi = x.astype(np.float16)
    lo = (x - hi.astype(np.float32)) * RSC
    return hi, hi.astype(ml_dtypes.float8_e4m3), lo.astype(ml_dtypes.float8_e4m3)


def kernel(query, encoder_outputs, src_lengths, W_in, W_out, b_out):
    query = np.asarray(query, dtype=np.float32)
    encoder_outputs = np.ascontiguousarray(np.asarray(encoder_outputs, np.float32))
    src_lengths = np.asarray(src_lengths)
    W_in = np.asarray(W_in, dtype=np.float32)
    W_out = np.asarray(W_out, dtype=np.float32)
    b_out = np.asarray(b_out, dtype=np.float32)

    # --- shared (weight) inputs ---
    W_inT = np.ascontiguousarray(W_in.T)                    # [h, o]
    Wihi, Wih8, Wil8 = _split16(W_inT)
    Wi8 = np.ascontiguousarray(np.stack([Wil8, Wih8], axis=0))  # stat: [lo, hi]
    Wqf = np.ascontiguousarray(W_out[:, :H].T).astype(np.float16)
    Wcf = np.ascontiguousarray(W_out[:, H:].T).astype(np.float16)
    bo = np.ascontiguousarray(b_out.reshape(HT, P).T)       # [p, m]

    # --- per-core shards ---
    in_maps = []
    for c in range(NCORES):
        bs = slice(c * BPC, (c + 1) * BPC)
        q = query[bs]                                       # [BPC, T, H]
        encs = encoder_outputs[bs]                          # [BPC, S, H]
        lens = np.asarray(src_lengths[bs], dtype=np.int64)

        qTa = np.ascontiguousarray(q.transpose(0, 2, 1))    # [BPC, H, T]
        qh, qh8, ql8 = _split16(qTa)
        q8a = np.ascontiguousarray(np.stack([qh8, ql8], axis=1))  # mov: [hi, lo]
        eTa = np.ascontiguousarray(encs.transpose(0, 2, 1))  # [BPC, H, S]
        eh, eh8, el8 = _split16(eTa)
        eT8a = np.ascontiguousarray(np.stack([el8, eh8], axis=1))  # stat: [lo, hi]

        maskca = np.zeros((BPC, P, ST), dtype=np.float32)
        pos = (np.arange(ST)[None, :] * P + np.arange(P)[:, None])  # [P, ST]
        for j in range(BPC):
            maskca[j][pos >= lens[j]] = MASKVAL

        in_maps.append({
            "qhi": qh, "q8": q8a, "eThi": eh, "eT8": eT8a,
            "encf": encs.astype(np.float16),
            "maskc": maskca, "Wihi": Wihi, "Wi8": Wi8,
            "Wqf": Wqf, "Wcf": Wcf, "bo": bo,
        })

    if not _nc_cache:
        _nc_cache.append(_build_nc())
    nc = _nc_cache[0]

    res = run_bass_kernel_spmd(nc, in_maps, core_ids=list(range(NCORES)))
    global LAST_RESULTS
    LAST_RESULTS = res

    out = np.empty((B, T, H), dtype=np.float32)
    for c in range(NCORES):
        o = res.results[c]["outT"]                          # [BPC, H, T]
        out[c * BPC:(c + 1) * BPC] = o.transpose(0, 2, 1)
    return out
